# revision 78
# baseline (speedup 1.0000x reference)
"""Multi-head attention (B=2, S=2048, D=1024, H=16, dk=64) on 8 Trainium2 NeuronCores.

Sharding: core c = (batch b = c//4, head-group g = c%4); each core handles one
batch and 4 heads (256 of the 1024 projection columns).  Output projection is
row-parallel (Megatron): each core emits a partial [2048, 1024] which the host
sums (the v/o biases fold into a host-side constant, q bias and the 1/sqrt(dk)
scale fold into Wq/bq, and the k bias is softmax-invariant and dropped).

All matmul operands are fp16 (1 PE cycle/row at any moving width, half the HBM
traffic of f32); PSUM accumulation stays f32.  HBM layouts are partition-major
so every DMA line is one long contiguous run.

Per-core dataflow:
  qT/kT projections land column-major [col, token], v token-major [token, col].
  Scores are computed transposed (S^T[k, q]) so the exp output P^T feeds the
  attn-V matmul as the STATIONARY operand ([128k x 128q] tiles) with v
  ([128k x 65]) as the short moving operand - 65 PE rows per tile instead of
  512.  The softmax denominator rides along as a ones column appended to v.
  The four per-q-subtile accumulation groups share one PSUM bank, and the PSUM
  start flag zeroes the whole bank, so the accumulators are memset once and
  every attn-V matmul accumulates (start=False).  Attention output lands
  token-major; it is normalized by the per-token reciprocal (a per-partition
  scalar = native broadcast) and transposed back to column-major by the DMA
  xbar (free on the PE); the last chunk uses PE transposes to dodge the xbar
  latency in the drain.

The emission schedule software-pipelines the whole kernel around the
activation engine (exp), which is the secondary wall after the PE: the
attention q-chunk loop is the master loop; projection and output-projection
work is injected between attention units right after each exp so filler never
delays the next scores; attn-V and per-pass normalize/transpose closures run a
few units behind their exp via a carried queue, which also absorbs late v
tiles and keeps pass boundaries from stalling the exp stream.
"""

import numpy as np

import concourse.bacc as bacc
import concourse.mybir as mybir
import concourse.tile as tile
from concourse.bass_utils import run_bass_kernel_spmd

F32 = mybir.dt.float32
F16 = mybir.dt.float16
EXP = mybir.ActivationFunctionType.Exp

B = 2          # batches
S = 2048       # sequence length
D = 1024       # d_model
DK = 64        # head dim
GROUPS = 4     # head-groups -> 8 cores = B * GROUPS
HG = 4         # heads per core
CC = HG * DK   # 256 projection columns per core
P = 128
KC = D // P    # 8 contraction chunks for projections
NQ5 = S // 512  # 4 q-chunks of 512
NKT = S // P    # 16 k-token chunks of 128
CT = CC // P    # 2 column-tiles (head-pairs) per core

_CACHE = {}


def _build_nc():
    nc = bacc.Bacc("TRN2", target_bir_lowering=False, debug=False, num_devices=8)

    # activation / weight layouts are partition-major on the host so each
    # SBUF partition's slice is one contiguous HBM run (few DMA descriptors)
    xq = nc.dram_tensor("xq", [P, NQ5, KC, 512], F16, kind="ExternalInput")
    xk = nc.dram_tensor("xk", [P, NQ5, KC, 512], F16, kind="ExternalInput")
    xv = nc.dram_tensor("xv", [P, NQ5, KC, 512], F16, kind="ExternalInput")
    wq = nc.dram_tensor("wq", [P, KC, CC], F16, kind="ExternalInput")
    wk = nc.dram_tensor("wk", [P, KC, CC], F16, kind="ExternalInput")
    wv = nc.dram_tensor("wv", [P, KC, CC], F16, kind="ExternalInput")
    wo = nc.dram_tensor("wo", [P, CT, D], F16, kind="ExternalInput")
    bq = nc.dram_tensor("bq", [P, CT], F32, kind="ExternalInput")
    ident = nc.dram_tensor("ident", [P, P], F16, kind="ExternalInput")
    out = nc.dram_tensor("out", [S, D], F16, kind="ExternalOutput")

    xq_v = xq.ap()
    xk_v = xk.ap()
    xv_v = xv.ap()
    out_v = out.ap().rearrange("(t p) n -> p t n", p=P)

    with tile.TileContext(nc) as tc:
        with (
            tc.tile_pool(name="wpool", bufs=1) as wpool,
            tc.tile_pool(name="kv", bufs=4) as kv,
            tc.tile_pool(name="xin", bufs=9) as xin,
            tc.tile_pool(name="qt", bufs=3) as qt_pool,
            tc.tile_pool(name="ptt", bufs=16) as ptt_pool,
            tc.tile_pool(name="atn", bufs=2) as atn_pool,
            tc.tile_pool(name="comb", bufs=2) as comb_pool,
            tc.tile_pool(name="outs", bufs=4) as outs_pool,
            tc.tile_pool(name="small", bufs=4) as small,
            tc.tile_pool(name="mm_ps", bufs=2, space="PSUM") as mm_ps,
            tc.tile_pool(name="st_ps", bufs=2, space="PSUM") as st_ps,
            tc.tile_pool(name="at_ps", bufs=1, space="PSUM") as at_ps,
        ):
            # ---- weights to SBUF ----
            wq_sb = wpool.tile([P, KC, CC], F16, tag="wq")
            wk_sb = wpool.tile([P, KC, CC], F16, tag="wk")
            wv_sb = wpool.tile([P, KC, CC], F16, tag="wv")
            wo_sb = wpool.tile([P, CT, D], F16, tag="wo")
            bq_sb = wpool.tile([P, CT], F32, tag="bq")

            # warm the ACT exp table while the first DMAs stream
            warm = wpool.tile([1, 1], F32, tag="warm")
            warm16 = wpool.tile([1, 1], F16, tag="warm16")
            nc.vector.memset(warm[:], 0.0)
            nc.scalar.activation(warm16[:], warm[:], EXP)

            # ---- persistent activations ----
            kT_t = [kv.tile([P, CT, 512], F16, tag="kT", name=f"kT{i}") for i in range(NQ5)]
            v_t = [kv.tile([P, 4, HG * (DK + 1)], F16, tag="v", name=f"v{i}") for i in range(NQ5)]
            for t5 in range(NQ5):
                for h in range(HG):
                    nc.gpsimd.memset(v_t[t5][:, :, h * (DK + 1) + DK], 1.0)

            # ---- input DMA emission (SP queue order == emission order) ----
            xts = {}

            def emit_x_dma(which, t5, split=0):
                src = {"k": xk_v, "q": xq_v, "v": xv_v}[which]
                t = xin.tile([P, KC, 512], F16, tag="xin", name=f"x{which}{t5}")
                if split == 2:
                    nc.sync.dma_start(t[:, 0:4], src[:, t5, 0:4])
                    nc.sync.dma_start(t[:, 4:8], src[:, t5, 4:8])
                else:
                    nc.sync.dma_start(t[:], src[:, t5])
                xts[which, t5] = t

            # head: moderately-sized pieces -- each dma_start pays ~625ns of
            # serialized HWDGE overhead, so pieces below ~1.5us of transfer
            # time cost more than they save.  k and q streams are interleaved
            # so the first scores matmul (which needs both) starts earliest.
            wk_v = wk.ap()
            xk0 = xin.tile([P, KC, 512], F16, tag="xin", name="xk0")
            xts["k", 0] = xk0
            xq0 = xin.tile([P, KC, 512], F16, tag="xin", name="xq0")
            xts["q", 0] = xq0
            nc.sync.dma_start(wk_sb[:, 0:4], wk_v[:, 0:4])
            nc.sync.dma_start(xk0[:, 0:3], xk_v[:, 0, 0:3])
            nc.sync.dma_start(wk_sb[:, 4:8], wk_v[:, 4:8])
            nc.sync.dma_start(xk0[:, 3:6], xk_v[:, 0, 3:6])
            nc.sync.dma_start(xk0[:, 6:8], xk_v[:, 0, 6:8])
            nc.sync.dma_start(wq_sb[:], wq.ap())
            nc.sync.dma_start(xq0[:, 0:4], xq_v[:, 0, 0:4])
            nc.sync.dma_start(xq0[:, 4:8], xq_v[:, 0, 4:8])
            nc.sync.dma_start(bq_sb[:], bq.ap())
            emit_x_dma("k", 1, split=2)
            id_sb = wpool.tile([P, P], F16, tag="ident")
            nc.sync.dma_start(id_sb[:], ident.ap())
            nc.sync.dma_start(wv_sb[:], wv.ap())
            xv0 = xin.tile([P, KC, 512], F16, tag="xin", name="xv0")
            xts["v", 0] = xv0
            nc.sync.dma_start(xv0[:, 0:4], xv_v[:, 0, 0:4])
            nc.sync.dma_start(xv0[:, 4:8], xv_v[:, 0, 4:8])

            def emit_wo_dma():
                nc.sync.dma_start(wo_sb[:], wo.ap())

            # ---- projection emitters (PE work; copies go to Pool/DVE) ----
            _half_ps = {}

            def emit_kproj_half(t5, ct, h):
                # half-granular version for injection into ACT-bound regions
                xkt = xts["k", t5]
                if h == 0:
                    _half_ps["k", t5, ct] = mm_ps.tile([P, 512], F32, tag="mm", name=f"kps{t5}{ct}")
                ps = _half_ps["k", t5, ct]
                for kc in range(4 * h, 4 * h + 4):
                    nc.tensor.matmul(
                        ps[:], wk_sb[:, kc, ct * P:(ct + 1) * P], xkt[:, kc],
                        start=(kc == 0), stop=(kc == KC - 1),
                    )
                if h == 1:
                    nc.vector.tensor_copy(kT_t[t5][:, ct, :], ps[:])

            def emit_kproj_ct(t5, ct):
                emit_kproj_half(t5, ct, 0)
                emit_kproj_half(t5, ct, 1)

            def emit_vproj_tt(t5, tt):
                xvt = xts["v", t5]
                ps = mm_ps.tile([P, 512], F32, tag="mm", name=f"vps{t5}{tt}")
                for kc in range(KC):
                    nc.tensor.matmul(
                        ps[:, :CC], xvt[:, kc, tt * P:(tt + 1) * P], wv_sb[:, kc],
                        start=(kc == 0), stop=(kc == KC - 1),
                    )
                dst = v_t[t5][:, tt, :].rearrange("p (h u) -> p h u", u=DK + 1)[:, :, :DK]
                src = ps[:, :CC].rearrange("p (h u) -> p h u", u=DK)
                nc.vector.tensor_copy(dst, src)

            qts = [None] * NQ5

            def emit_qproj_half(q5, ct, h):
                xqt = xts["q", q5]
                if ct == 0 and h == 0:
                    qts[q5] = qt_pool.tile([P, CT, 512], F16, tag="qt", name=f"qt{q5}")
                qt = qts[q5]
                if h == 0:
                    _half_ps["q", q5, ct] = mm_ps.tile([P, 512], F32, tag="mm", name=f"qps{q5}{ct}")
                ps = _half_ps["q", q5, ct]
                for kc in range(4 * h, 4 * h + 4):
                    nc.tensor.matmul(
                        ps[:], wq_sb[:, kc, ct * P:(ct + 1) * P], xqt[:, kc],
                        start=(kc == 0), stop=(kc == KC - 1),
                    )
                if h == 1:
                    nc.vector.tensor_scalar_add(qt[:, ct], ps[:], bq_sb[:, ct:ct + 1])

            def emit_qproj_ct(q5, ct):
                emit_qproj_half(q5, ct, 0)
                emit_qproj_half(q5, ct, 1)

            # ---- output projection piece: one 128-token tile, one 512-col half ----
            outs = {}

            _tail_tags = ["mm", "at0", "at1", "mm"]

            def emit_outproj_piece(q5, tt, n2, last=False):
                comb = combs[q5]
                if n2 == 0:
                    outs[q5, tt] = outs_pool.tile([P, D], F16, tag="out", name=f"out{q5}_{tt}")
                out_t = outs[q5, tt]
                # in the tail the at-accumulator banks are free: rotate psum
                # across 4 slots so the matmul stream is not copy-latency-gated
                tag = _tail_tags[(2 * tt + n2) % 4] if last else "mm"
                pool = at_ps if tag.startswith("at") else mm_ps
                ps = pool.tile([P, 512], F32, tag=tag, name=f"ops{q5}{tt}{n2}")
                for c2 in range(CT):
                    nc.tensor.matmul(
                        ps[:], comb[:, c2, tt * P:(tt + 1) * P],
                        wo_sb[:, c2, n2 * 512:(n2 + 1) * 512],
                        start=(c2 == 0), stop=(c2 == CT - 1),
                    )
                if last:
                    # keep DVE free for the trailing normalizes: stage halves
                    # on ACT and Pool, which are both idle at the tail
                    if n2 == 0:
                        nc.scalar.copy(out_t[:, 0:512], ps[:])
                    else:
                        nc.vector.tensor_copy(out_t[:, 512:D], ps[:])
                        nc.sync.dma_start(out_v[:, q5 * 4 + tt, :], out_t[:])
                else:
                    nc.vector.tensor_copy(out_t[:, n2 * 512:(n2 + 1) * 512], ps[:])
                    if n2 == 1:
                        nc.sync.dma_start(out_v[:, q5 * 4 + tt, :], out_t[:])

            # ---- attention ----
            combs = [None] * NQ5

            pend = []  # deferred PE/cleanup closures, carried across passes

            def attention(q5, inj):
                """inj: dict unit-index -> list of closures to emit at that unit
                (before that unit's scores).  Unit = hp*16+kc.  attn-V and the
                per-pass normalize/transpose run ~2 units behind their exp via
                the carried `pend` queue, so pass boundaries never delay the
                next scores/exp pair."""
                qt = qts[q5]
                comb = comb_pool.tile([P, CT, 512], F16, tag="comb", name=f"comb{q5}")
                combs[q5] = comb
                for hp in range(CT):
                    # the PSUM start flag zeroes the whole bank, so the four
                    # interleaved per-qq accumulation groups cannot each use
                    # start=True: memset once and always accumulate
                    atj = []
                    for j in range(2):
                        at = at_ps.tile([P, 4, DK + 1], F32, tag=f"at{j}",
                                        name=f"at{q5}_{hp}{j}")
                        nc.vector.memset(at[:], 0.0)
                        atj.append(at)
                    for kc in range(NKT):
                        t5, tt = kc // 4, kc % 4
                        st = st_ps.tile([P, 2, 512], F32, tag="st", name=f"st{q5}_{hp}{kc}")
                        for j in range(2):
                            r = DK * j
                            nc.tensor.matmul(
                                st[:, j],
                                kT_t[t5][r:r + DK, hp, tt * P:(tt + 1) * P],
                                qt[r:r + DK, hp],
                                start=True, stop=True,
                            )
                        ptt = ptt_pool.tile([P, 2, 512], F16, tag="ptt", name=f"pt{q5}_{hp}{kc}")
                        nc.scalar.activation(ptt[:], st[:], EXP)
                        for f in inj.get(hp * NKT + kc, ()):
                            # injected after this unit's exp so filler work
                            # never delays the next scores/exp pair
                            f()
                        while len(pend) > 3:
                            pend.pop(0)()

                        def make_attnv(kc=kc, t5=t5, tt=tt, ptt=ptt, atj=atj, hp=hp):
                            def go():
                                for j in range(2):
                                    h = 2 * hp + j
                                    vmov = v_t[t5][:, tt, h * (DK + 1):(h + 1) * (DK + 1)]
                                    for qq in range(4):
                                        nc.tensor.matmul(
                                            atj[j][:, qq],
                                            ptt[:, j, qq * P:(qq + 1) * P],
                                            vmov,
                                            start=False, stop=(kc == NKT - 1),
                                            skip_group_check=True,
                                        )
                            return go

                        pend.append(make_attnv())

                    def make_fin(q5=q5, hp=hp, atj=atj, comb=comb,
                                 last=(q5 == NQ5 - 1 and hp == CT - 1)):
                        def fin():
                            # normalize (token-major) + DMA-transpose to col-major
                            atn = atn_pool.tile([P, 4, 2, DK], F16, tag="atn",
                                                name=f"atn{q5}_{hp}")
                            for j in range(2):
                                rc = small.tile([P, 4, 1], F32, tag="rc",
                                                name=f"rc{q5}_{hp}{j}")
                                nc.vector.reciprocal(rc[:, :, 0], atj[j][:, :, DK])
                                nc.vector.tensor_mul(
                                    atn[:, :, j], atj[j][:, :, :DK],
                                    rc.broadcast_to([P, 4, DK]),
                                )
                            if not last:
                                for qq in range(4):
                                    nc.sync.dma_start_transpose(
                                        comb[:, hp, qq * P:(qq + 1) * P], atn[:, qq]
                                    )
                            else:
                                # tail: PE transposes via identity avoid the
                                # ~2.4us DMA xbar latency right at the end
                                for qq in range(4):
                                    psT = mm_ps.tile(
                                        [P, 512], F32, tag="mm",
                                        name=f"psT{qq}").bitcast(F16)
                                    nc.tensor.transpose(
                                        psT[:, 0:P], atn[:, qq], id_sb[:])
                                    nc.vector.tensor_copy(
                                        comb[:, hp, qq * P:(qq + 1) * P],
                                        psT[:, 0:P])
                        return fin

                    pend.append(make_fin())

            # ---- master schedule ----
            emit_kproj_ct(0, 0)
            emit_qproj_ct(0, 0)

            def mk(f, *a):
                def go():
                    f(*a)
                return go

            def mk2(f, *a):
                # x-DMA prefetch bundled with a PE piece
                which, t5, g, ga = f, a[0], a[1], a[2:]

                def go():
                    emit_x_dma(which, t5)
                    g(*ga)
                return go

            # att(0): k/v projections for t5=1..3 injected at their deadlines
            # (scores at unit 4*t5 need kT[t5]; attn-V for kc -- emitted with a
            # 2-unit lag -- needs v[kc//4][kc%4] by unit kc+2).
            inj0 = {
                2: [mk(emit_x_dma, "k", 2, 2)],
                3: [mk(emit_kproj_ct, 1, 0), mk(emit_vproj_tt, 0, 0)],
                4: [mk(emit_x_dma, "k", 3, 2), mk(emit_vproj_tt, 0, 1)],
                5: [mk(emit_kproj_half, 0, 1, 0), mk(emit_vproj_tt, 0, 2)],
                6: [mk2("v", 1, emit_kproj_half, 0, 1, 1), mk(emit_vproj_tt, 0, 3)],
                7: [mk(emit_kproj_ct, 2, 0), mk(emit_qproj_half, 0, 1, 0)],
                8: [mk(emit_vproj_tt, 1, 0), mk(emit_qproj_half, 0, 1, 1)],
                9: [mk2("v", 2, emit_vproj_tt, 1, 1)],
                10: [mk(emit_vproj_tt, 1, 2)],
                11: [mk(emit_kproj_ct, 3, 0), mk(emit_vproj_tt, 1, 3)],
                12: [mk2("v", 3, emit_vproj_tt, 2, 0)],
                13: [mk(emit_vproj_tt, 2, 1), mk(emit_x_dma, "q", 1), mk(emit_wo_dma)],
                14: [mk(emit_vproj_tt, 2, 2)],
                15: [mk(emit_vproj_tt, 2, 3), mk(emit_vproj_tt, 3, 0)],
                16: [mk(emit_vproj_tt, 3, 1), mk(emit_kproj_half, 1, 1, 0)],
                17: [mk(emit_vproj_tt, 3, 2), mk(emit_x_dma, "q", 2)],
                18: [mk(emit_kproj_half, 1, 1, 1), mk(emit_vproj_tt, 3, 3)],
                20: [mk(emit_kproj_half, 2, 1, 0)],
                22: [mk(emit_kproj_half, 2, 1, 1)],
                24: [mk(emit_kproj_half, 3, 1, 0)],
                26: [mk(emit_kproj_half, 3, 1, 1)],
                27: [mk(emit_qproj_half, 1, 0, 0)],
                28: [mk(emit_qproj_half, 1, 0, 1)],
                29: [mk(emit_qproj_half, 1, 1, 0)],
                30: [mk(emit_qproj_half, 1, 1, 1)],
            }
            attention(0, inj0)

            inj1 = {
                17: [mk(emit_x_dma, "q", 3)],
                26: [mk(emit_qproj_half, 2, 0, 0)],
                28: [mk(emit_qproj_half, 2, 0, 1)],
                29: [mk(emit_qproj_half, 2, 1, 0)],
                30: [mk(emit_qproj_half, 2, 1, 1)],
            }
            for i, (tt, n2) in enumerate((t, n) for t in range(4) for n in range(2)):
                inj1.setdefault(4 + 3 * i, []).append(mk(emit_outproj_piece, 0, tt, n2))
            attention(1, inj1)

            inj2 = {
                26: [mk(emit_qproj_half, 3, 0, 0)],
                28: [mk(emit_qproj_half, 3, 0, 1)],
                29: [mk(emit_qproj_half, 3, 1, 0)],
                30: [mk(emit_qproj_half, 3, 1, 1)],
            }
            for i, (tt, n2) in enumerate((t, n) for t in range(4) for n in range(2)):
                inj2.setdefault(4 + 3 * i, []).append(mk(emit_outproj_piece, 1, tt, n2))
            attention(2, inj2)

            inj3 = {}
            for i, (tt, n2) in enumerate((t, n) for t in range(4) for n in range(2)):
                inj3.setdefault(4 + 3 * i, []).append(mk(emit_outproj_piece, 2, tt, n2))
            attention(3, inj3)

            for f in pend:
                f()
            for tt in range(4):
                for n2 in range(2):
                    emit_outproj_piece(3, tt, n2, last=True)

    nc.compile()
    return nc


def _get_nc():
    if "nc" not in _CACHE:
        _CACHE["nc"] = _build_nc()
    return _CACHE["nc"]


def kernel(query, key, value, Wq, bq, Wk, bk, Wv, bv, Wo, bo):
    nc = _get_nc()
    scale = np.float32(1.0 / np.sqrt(DK))

    query = np.asarray(query, dtype=np.float32)
    key = np.asarray(key, dtype=np.float32)
    value = np.asarray(value, dtype=np.float32)
    Wq = np.asarray(Wq, dtype=np.float32)
    Wk = np.asarray(Wk, dtype=np.float32)
    Wv = np.asarray(Wv, dtype=np.float32)
    Wo = np.asarray(Wo, dtype=np.float32)

    def xfmt(x):
        # [S, D] -> [P, NQ5, KC, 512]: partition-major, one contiguous run
        # per (partition, t5) pair
        return np.ascontiguousarray(
            x.T.reshape(KC, P, NQ5, 512).transpose(1, 2, 0, 3).astype(np.float16))

    xq_np = [xfmt(query[b]) for b in range(B)]
    xk_np = [xfmt(key[b]) for b in range(B)]
    xv_np = [xfmt(value[b]) for b in range(B)]

    wq_np, wk_np, wv_np, wo_np, bq_np = [], [], [], [], []
    for g in range(GROUPS):
        gsl = slice(CC * g, CC * (g + 1))
        def wfmt(w):
            # [D, CC] -> [P, KC, CC]
            return np.ascontiguousarray(
                w.reshape(KC, P, CC).transpose(1, 0, 2).astype(np.float16))

        wq_np.append(wfmt((Wq[gsl] * scale).T))
        wk_np.append(wfmt(Wk[gsl].T))
        wv_np.append(wfmt(Wv[gsl].T))
        wo_np.append(np.ascontiguousarray(
            Wo[:, gsl].T.reshape(CT, P, D).transpose(1, 0, 2).astype(np.float16)))
        bq_np.append(np.ascontiguousarray((np.asarray(bq, np.float32)[gsl] * scale).reshape(CT, P).T))

    ident_np = np.eye(P, dtype=np.float16)
    in_maps = []
    for c in range(8):
        b, g = c // GROUPS, c % GROUPS
        in_maps.append({
            "xq": xq_np[b], "xk": xk_np[b], "xv": xv_np[b],
            "wq": wq_np[g], "wk": wk_np[g], "wv": wv_np[g],
            "wo": wo_np[g], "bq": bq_np[g], "ident": ident_np,
        })

    res = None
    for attempt in range(3):
        try:
            res = run_bass_kernel_spmd(nc, in_maps, core_ids=list(range(8)))
            _CACHE["last_res"] = res
            break
        except Exception:
            # transient NRT_EXEC_UNIT_UNRECOVERABLE wedge: tear down the PJRT
            # client and retry with a fresh backend connection
            if attempt == 2:
                raise
            import time
            time.sleep(15)
            try:
                import jax
                jax.clear_backends()
            except Exception:
                try:
                    from jax._src import xla_bridge
                    xla_bridge.backends.cache_clear()
                except Exception:
                    pass

    # host combine: sum the 4 head-group partials per batch, add folded bias
    bias = (np.asarray(bo, np.float64)
            + np.asarray(Wo, np.float64) @ np.asarray(bv, np.float64)).astype(np.float32)
    out = np.empty((B, S, D), dtype=np.float32)
    for b in range(B):
        acc = res.results[b * GROUPS + 0]["out"].astype(np.float32)
        for g in range(1, GROUPS):
            acc = acc + res.results[b * GROUPS + g]["out"]
        out[b] = acc + bias
    return out


# revision 82
# speedup vs baseline: 1.0525x; 1.0525x over previous
"""Multi-head attention (B=2, S=2048, D=1024, H=16, dk=64) on 8 Trainium2 NeuronCores.

Sharding: core c = (batch b = c//4, head-group g = c%4); each core handles one
batch and 4 heads (256 of the 1024 projection columns).  Output projection is
row-parallel (Megatron): each core emits a partial [2048, 1024] which the host
sums (the v/o biases fold into a host-side constant, q bias and the 1/sqrt(dk)
scale fold into Wq/bq, and the k bias is softmax-invariant and dropped).

All matmul operands are fp16 (1 PE cycle/row at any moving width, half the HBM
traffic of f32); PSUM accumulation stays f32.  HBM layouts are partition-major
so every DMA line is one long contiguous run.

Per-core dataflow:
  qT/kT projections land column-major [col, token], v token-major [token, col].
  Scores are computed transposed (S^T[k, q]) so the exp output P^T feeds the
  attn-V matmul as the STATIONARY operand ([128k x 128q] tiles) with v
  ([128k x 65]) as the short moving operand - 65 PE rows per tile instead of
  512.  The softmax denominator rides along as a ones column appended to v.
  The four per-q-subtile accumulation groups share one PSUM bank, and the PSUM
  start flag zeroes the whole bank, so the accumulators are memset once and
  every attn-V matmul accumulates (start=False).  Attention output lands
  token-major; it is normalized by the per-token reciprocal (a per-partition
  scalar = native broadcast) and transposed back to column-major by the DMA
  xbar (free on the PE); the last chunk uses PE transposes to dodge the xbar
  latency in the drain.

The emission schedule software-pipelines the whole kernel around the
activation engine (exp), which is the secondary wall after the PE: the
attention q-chunk loop is the master loop; projection and output-projection
work is injected between attention units right after each exp so filler never
delays the next scores; attn-V and per-pass normalize/transpose closures run a
few units behind their exp via a carried queue, which also absorbs late v
tiles and keeps pass boundaries from stalling the exp stream.
"""

import numpy as np

import concourse.bacc as bacc
import concourse.mybir as mybir
import concourse.tile as tile
from concourse.bass_utils import run_bass_kernel_spmd

F32 = mybir.dt.float32
F16 = mybir.dt.float16
F8 = mybir.dt.float8e4
DR = mybir.MatmulPerfMode.DoubleRow
EXP = mybir.ActivationFunctionType.Exp

B = 2          # batches
S = 2048       # sequence length
D = 1024       # d_model
DK = 64        # head dim
GROUPS = 4     # head-groups -> 8 cores = B * GROUPS
HG = 4         # heads per core
CC = HG * DK   # 256 projection columns per core
P = 128
KC = D // P    # 8 contraction chunks for projections
NQ5 = S // 512  # 4 q-chunks of 512
NKT = S // P    # 16 k-token chunks of 128
CT = CC // P    # 2 column-tiles (head-pairs) per core

_CACHE = {}


def _build_nc():
    nc = bacc.Bacc("TRN2", target_bir_lowering=False, debug=False, num_devices=8)

    # activation / weight layouts are partition-major on the host so each
    # SBUF partition's slice is one contiguous HBM run (few DMA descriptors)
    # q/k projections run in fp8 with the DoubleRow perf mode (2 reduction
    # tiles per pass, 0.5 PE cycles/row): inputs and weights are pre-split on
    # the host into [64, ..., 2, n] (two 64-row halves of each 128-row D chunk)
    xq = nc.dram_tensor("xq", [64, NQ5, KC, 2, 512], F8, kind="ExternalInput")
    xk = nc.dram_tensor("xk", [64, NQ5, KC, 2, 512], F8, kind="ExternalInput")
    xv = nc.dram_tensor("xv", [P, NQ5, KC, 512], F16, kind="ExternalInput")
    wq = nc.dram_tensor("wq", [64, KC, 2, CC], F8, kind="ExternalInput")
    wk = nc.dram_tensor("wk", [64, KC, 2, CC], F8, kind="ExternalInput")
    wv = nc.dram_tensor("wv", [P, KC, CC], F16, kind="ExternalInput")
    wo = nc.dram_tensor("wo", [P, CT, D], F16, kind="ExternalInput")
    bq = nc.dram_tensor("bq", [P, CT], F32, kind="ExternalInput")
    ident = nc.dram_tensor("ident", [P, P], F16, kind="ExternalInput")
    out = nc.dram_tensor("out", [S, D], F16, kind="ExternalOutput")

    xq_v = xq.ap()
    xk_v = xk.ap()
    xv_v = xv.ap()
    out_v = out.ap().rearrange("(t p) n -> p t n", p=P)

    with tile.TileContext(nc) as tc:
        with (
            tc.tile_pool(name="wpool", bufs=1) as wpool,
            tc.tile_pool(name="kv", bufs=4) as kv,
            tc.tile_pool(name="xin", bufs=9) as xin,
            tc.tile_pool(name="qt", bufs=3) as qt_pool,
            tc.tile_pool(name="ptt", bufs=16) as ptt_pool,
            tc.tile_pool(name="atn", bufs=2) as atn_pool,
            tc.tile_pool(name="comb", bufs=2) as comb_pool,
            tc.tile_pool(name="outs", bufs=4) as outs_pool,
            tc.tile_pool(name="small", bufs=4) as small,
            tc.tile_pool(name="mm_ps", bufs=2, space="PSUM") as mm_ps,
            tc.tile_pool(name="st_ps", bufs=2, space="PSUM") as st_ps,
            tc.tile_pool(name="at_ps", bufs=1, space="PSUM") as at_ps,
        ):
            # ---- weights to SBUF ----
            wq_sb = wpool.tile([64, KC, 2, CC], F8, tag="wq")
            wk_sb = wpool.tile([64, KC, 2, CC], F8, tag="wk")
            wv_sb = wpool.tile([P, KC, CC], F16, tag="wv")
            wo_sb = wpool.tile([P, CT, D], F16, tag="wo")
            bq_sb = wpool.tile([P, CT], F32, tag="bq")

            # warm the ACT exp table while the first DMAs stream
            warm = wpool.tile([1, 1], F32, tag="warm")
            warm16 = wpool.tile([1, 1], F16, tag="warm16")
            nc.vector.memset(warm[:], 0.0)
            nc.scalar.activation(warm16[:], warm[:], EXP)

            # ---- persistent activations ----
            kT_t = [kv.tile([P, CT, 512], F16, tag="kT", name=f"kT{i}") for i in range(NQ5)]
            v_t = [kv.tile([P, 4, HG * (DK + 1)], F16, tag="v", name=f"v{i}") for i in range(NQ5)]
            for t5 in range(NQ5):
                for h in range(HG):
                    nc.gpsimd.memset(v_t[t5][:, :, h * (DK + 1) + DK], 1.0)

            # ---- input DMA emission (SP queue order == emission order) ----
            xts = {}

            def emit_x_dma(which, t5, split=0):
                src = {"k": xk_v, "q": xq_v, "v": xv_v}[which]
                if which == "v":
                    t = xin.tile([P, KC, 512], F16, tag="xin", name=f"x{which}{t5}")
                else:
                    t = xin.tile([64, KC, 2, 512], F8, tag="xin", name=f"x{which}{t5}")
                if split == 2:
                    nc.sync.dma_start(t[:, 0:4], src[:, t5, 0:4])
                    nc.sync.dma_start(t[:, 4:8], src[:, t5, 4:8])
                else:
                    nc.sync.dma_start(t[:], src[:, t5])
                xts[which, t5] = t

            # head: moderately-sized pieces -- each dma_start pays ~625ns of
            # serialized HWDGE overhead, so pieces below ~1.5us of transfer
            # time cost more than they save.  k and q streams are interleaved
            # so the first scores matmul (which needs both) starts earliest.
            wk_v = wk.ap()
            xk0 = xin.tile([64, KC, 2, 512], F8, tag="xin", name="xk0")
            xts["k", 0] = xk0
            xq0 = xin.tile([64, KC, 2, 512], F8, tag="xin", name="xq0")
            xts["q", 0] = xq0
            nc.sync.dma_start(wk_sb[:, 0:4], wk_v[:, 0:4])
            nc.sync.dma_start(xk0[:, 0:3], xk_v[:, 0, 0:3])
            nc.sync.dma_start(wk_sb[:, 4:8], wk_v[:, 4:8])
            nc.sync.dma_start(xk0[:, 3:6], xk_v[:, 0, 3:6])
            nc.sync.dma_start(xk0[:, 6:8], xk_v[:, 0, 6:8])
            nc.sync.dma_start(wq_sb[:], wq.ap())
            nc.sync.dma_start(xq0[:, 0:4], xq_v[:, 0, 0:4])
            nc.sync.dma_start(xq0[:, 4:8], xq_v[:, 0, 4:8])
            nc.sync.dma_start(bq_sb[:], bq.ap())
            emit_x_dma("k", 1, split=2)
            id_sb = wpool.tile([P, P], F16, tag="ident")
            nc.sync.dma_start(id_sb[:], ident.ap())
            nc.sync.dma_start(wv_sb[:], wv.ap())
            xv0 = xin.tile([P, KC, 512], F16, tag="xin", name="xv0")
            xts["v", 0] = xv0
            nc.sync.dma_start(xv0[:, 0:4], xv_v[:, 0, 0:4])
            nc.sync.dma_start(xv0[:, 4:8], xv_v[:, 0, 4:8])

            def emit_wo_dma():
                nc.sync.dma_start(wo_sb[:], wo.ap())

            # ---- projection emitters (PE work; copies go to Pool/DVE) ----
            _half_ps = {}

            def emit_kproj_half(t5, ct, h):
                # half-granular version for injection into ACT-bound regions
                xkt = xts["k", t5]
                if h == 0:
                    _half_ps["k", t5, ct] = mm_ps.tile([P, 512], F32, tag="mm", name=f"kps{t5}{ct}")
                ps = _half_ps["k", t5, ct]
                for kc in range(4 * h, 4 * h + 4):
                    nc.tensor.matmul(
                        ps[:], wk_sb[:, kc, :, ct * P:(ct + 1) * P], xkt[:, kc],
                        start=(kc == 0), stop=(kc == KC - 1), perf_mode=DR,
                    )
                if h == 1:
                    nc.vector.tensor_copy(kT_t[t5][:, ct, :], ps[:])

            def emit_kproj_ct(t5, ct):
                emit_kproj_half(t5, ct, 0)
                emit_kproj_half(t5, ct, 1)

            def emit_vproj_tt(t5, tt):
                xvt = xts["v", t5]
                ps = mm_ps.tile([P, 512], F32, tag="mm", name=f"vps{t5}{tt}")
                for kc in range(KC):
                    nc.tensor.matmul(
                        ps[:, :CC], xvt[:, kc, tt * P:(tt + 1) * P], wv_sb[:, kc],
                        start=(kc == 0), stop=(kc == KC - 1),
                    )
                dst = v_t[t5][:, tt, :].rearrange("p (h u) -> p h u", u=DK + 1)[:, :, :DK]
                src = ps[:, :CC].rearrange("p (h u) -> p h u", u=DK)
                nc.vector.tensor_copy(dst, src)

            qts = [None] * NQ5

            def emit_qproj_half(q5, ct, h):
                xqt = xts["q", q5]
                if ct == 0 and h == 0:
                    qts[q5] = qt_pool.tile([P, CT, 512], F16, tag="qt", name=f"qt{q5}")
                qt = qts[q5]
                if h == 0:
                    _half_ps["q", q5, ct] = mm_ps.tile([P, 512], F32, tag="mm", name=f"qps{q5}{ct}")
                ps = _half_ps["q", q5, ct]
                for kc in range(4 * h, 4 * h + 4):
                    nc.tensor.matmul(
                        ps[:], wq_sb[:, kc, :, ct * P:(ct + 1) * P], xqt[:, kc],
                        start=(kc == 0), stop=(kc == KC - 1), perf_mode=DR,
                    )
                if h == 1:
                    with nc.allow_low_precision(reason="fp16 qt"):
                        nc.vector.scalar_tensor_tensor(
                            qt[:, ct], ps[:], float(1.0 / np.sqrt(DK)),
                            bq_sb[:, ct:ct + 1].broadcast_to([P, 512]),
                            op0=mybir.AluOpType.mult, op1=mybir.AluOpType.add,
                        )

            def emit_qproj_ct(q5, ct):
                emit_qproj_half(q5, ct, 0)
                emit_qproj_half(q5, ct, 1)

            # ---- output projection piece: one 128-token tile, one 512-col half ----
            outs = {}

            _tail_tags = ["mm", "at0", "at1", "mm"]

            def emit_outproj_piece(q5, tt, n2, last=False):
                comb = combs[q5]
                if n2 == 0:
                    outs[q5, tt] = outs_pool.tile([P, D], F16, tag="out", name=f"out{q5}_{tt}")
                out_t = outs[q5, tt]
                # in the tail the at-accumulator banks are free: rotate psum
                # across 4 slots so the matmul stream is not copy-latency-gated
                tag = _tail_tags[(2 * tt + n2) % 4] if last else "mm"
                pool = at_ps if tag.startswith("at") else mm_ps
                ps = pool.tile([P, 512], F32, tag=tag, name=f"ops{q5}{tt}{n2}")
                for c2 in range(CT):
                    nc.tensor.matmul(
                        ps[:], comb[:, c2, tt * P:(tt + 1) * P],
                        wo_sb[:, c2, n2 * 512:(n2 + 1) * 512],
                        start=(c2 == 0), stop=(c2 == CT - 1),
                    )
                if last:
                    # keep DVE free for the trailing normalizes: stage halves
                    # on ACT and Pool, which are both idle at the tail
                    if n2 == 0:
                        nc.scalar.copy(out_t[:, 0:512], ps[:])
                    else:
                        nc.vector.tensor_copy(out_t[:, 512:D], ps[:])
                        nc.sync.dma_start(out_v[:, q5 * 4 + tt, :], out_t[:])
                else:
                    nc.vector.tensor_copy(out_t[:, n2 * 512:(n2 + 1) * 512], ps[:])
                    if n2 == 1:
                        nc.sync.dma_start(out_v[:, q5 * 4 + tt, :], out_t[:])

            # ---- attention ----
            combs = [None] * NQ5

            pend = []  # deferred PE/cleanup closures, carried across passes

            def attention(q5, inj):
                """inj: dict unit-index -> list of closures to emit at that unit
                (before that unit's scores).  Unit = hp*16+kc.  attn-V and the
                per-pass normalize/transpose run ~2 units behind their exp via
                the carried `pend` queue, so pass boundaries never delay the
                next scores/exp pair."""
                qt = qts[q5]
                comb = comb_pool.tile([P, CT, 512], F16, tag="comb", name=f"comb{q5}")
                combs[q5] = comb
                for hp in range(CT):
                    # the PSUM start flag zeroes the whole bank, so the four
                    # interleaved per-qq accumulation groups cannot each use
                    # start=True: memset once and always accumulate
                    atj = []
                    for j in range(2):
                        at = at_ps.tile([P, 4, DK + 1], F32, tag=f"at{j}",
                                        name=f"at{q5}_{hp}{j}")
                        nc.vector.memset(at[:], 0.0)
                        atj.append(at)
                    for kc in range(NKT):
                        t5, tt = kc // 4, kc % 4
                        st = st_ps.tile([P, 2, 512], F32, tag="st", name=f"st{q5}_{hp}{kc}")
                        for j in range(2):
                            r = DK * j
                            nc.tensor.matmul(
                                st[:, j],
                                kT_t[t5][r:r + DK, hp, tt * P:(tt + 1) * P],
                                qt[r:r + DK, hp],
                                start=True, stop=True,
                            )
                        ptt = ptt_pool.tile([P, 2, 512], F16, tag="ptt", name=f"pt{q5}_{hp}{kc}")
                        nc.scalar.activation(ptt[:], st[:], EXP)
                        for f in inj.get(hp * NKT + kc, ()):
                            # injected after this unit's exp so filler work
                            # never delays the next scores/exp pair
                            f()
                        while len(pend) > 3:
                            pend.pop(0)()

                        def make_attnv(kc=kc, t5=t5, tt=tt, ptt=ptt, atj=atj, hp=hp):
                            def go():
                                for j in range(2):
                                    h = 2 * hp + j
                                    vmov = v_t[t5][:, tt, h * (DK + 1):(h + 1) * (DK + 1)]
                                    for qq in range(4):
                                        nc.tensor.matmul(
                                            atj[j][:, qq],
                                            ptt[:, j, qq * P:(qq + 1) * P],
                                            vmov,
                                            start=False, stop=(kc == NKT - 1),
                                            skip_group_check=True,
                                        )
                            return go

                        pend.append(make_attnv())

                    def make_fin(q5=q5, hp=hp, atj=atj, comb=comb,
                                 last=(q5 == NQ5 - 1 and hp == CT - 1)):
                        def fin():
                            # normalize (token-major) + DMA-transpose to col-major
                            atn = atn_pool.tile([P, 4, 2, DK], F16, tag="atn",
                                                name=f"atn{q5}_{hp}")
                            for j in range(2):
                                rc = small.tile([P, 4, 1], F32, tag="rc",
                                                name=f"rc{q5}_{hp}{j}")
                                nc.vector.reciprocal(rc[:, :, 0], atj[j][:, :, DK])
                                nc.vector.tensor_mul(
                                    atn[:, :, j], atj[j][:, :, :DK],
                                    rc.broadcast_to([P, 4, DK]),
                                )
                            if not last:
                                for qq in range(4):
                                    nc.sync.dma_start_transpose(
                                        comb[:, hp, qq * P:(qq + 1) * P], atn[:, qq]
                                    )
                            else:
                                # tail: PE transposes via identity avoid the
                                # ~2.4us DMA xbar latency right at the end
                                for qq in range(4):
                                    psT = mm_ps.tile(
                                        [P, 512], F32, tag="mm",
                                        name=f"psT{qq}").bitcast(F16)
                                    nc.tensor.transpose(
                                        psT[:, 0:P], atn[:, qq], id_sb[:])
                                    nc.vector.tensor_copy(
                                        comb[:, hp, qq * P:(qq + 1) * P],
                                        psT[:, 0:P])
                        return fin

                    pend.append(make_fin())

            # ---- master schedule ----
            emit_kproj_ct(0, 0)
            emit_qproj_ct(0, 0)

            def mk(f, *a):
                def go():
                    f(*a)
                return go

            def mk2(f, *a):
                # x-DMA prefetch bundled with a PE piece
                which, t5, g, ga = f, a[0], a[1], a[2:]

                def go():
                    emit_x_dma(which, t5)
                    g(*ga)
                return go

            # att(0): k/v projections for t5=1..3 injected at their deadlines
            # (scores at unit 4*t5 need kT[t5]; attn-V for kc -- emitted with a
            # 2-unit lag -- needs v[kc//4][kc%4] by unit kc+2).
            inj0 = {
                2: [mk(emit_x_dma, "k", 2, 2)],
                3: [mk(emit_kproj_ct, 1, 0), mk(emit_vproj_tt, 0, 0)],
                4: [mk(emit_x_dma, "k", 3, 2), mk(emit_vproj_tt, 0, 1)],
                5: [mk(emit_kproj_half, 0, 1, 0), mk(emit_vproj_tt, 0, 2)],
                6: [mk2("v", 1, emit_kproj_half, 0, 1, 1), mk(emit_vproj_tt, 0, 3)],
                7: [mk(emit_kproj_ct, 2, 0), mk(emit_qproj_half, 0, 1, 0)],
                8: [mk(emit_vproj_tt, 1, 0), mk(emit_qproj_half, 0, 1, 1)],
                9: [mk2("v", 2, emit_vproj_tt, 1, 1)],
                10: [mk(emit_vproj_tt, 1, 2)],
                11: [mk(emit_kproj_ct, 3, 0), mk(emit_vproj_tt, 1, 3)],
                12: [mk2("v", 3, emit_vproj_tt, 2, 0)],
                13: [mk(emit_vproj_tt, 2, 1), mk(emit_x_dma, "q", 1), mk(emit_wo_dma)],
                14: [mk(emit_vproj_tt, 2, 2)],
                15: [mk(emit_vproj_tt, 2, 3), mk(emit_vproj_tt, 3, 0)],
                16: [mk(emit_vproj_tt, 3, 1), mk(emit_kproj_half, 1, 1, 0)],
                17: [mk(emit_vproj_tt, 3, 2), mk(emit_x_dma, "q", 2)],
                18: [mk(emit_kproj_half, 1, 1, 1), mk(emit_vproj_tt, 3, 3)],
                20: [mk(emit_kproj_half, 2, 1, 0)],
                22: [mk(emit_kproj_half, 2, 1, 1)],
                24: [mk(emit_kproj_half, 3, 1, 0)],
                26: [mk(emit_kproj_half, 3, 1, 1)],
                27: [mk(emit_qproj_half, 1, 0, 0)],
                28: [mk(emit_qproj_half, 1, 0, 1)],
                29: [mk(emit_qproj_half, 1, 1, 0)],
                30: [mk(emit_qproj_half, 1, 1, 1)],
            }
            attention(0, inj0)

            inj1 = {
                17: [mk(emit_x_dma, "q", 3)],
                26: [mk(emit_qproj_half, 2, 0, 0)],
                28: [mk(emit_qproj_half, 2, 0, 1)],
                29: [mk(emit_qproj_half, 2, 1, 0)],
                30: [mk(emit_qproj_half, 2, 1, 1)],
            }
            for i, (tt, n2) in enumerate((t, n) for t in range(4) for n in range(2)):
                inj1.setdefault(4 + 3 * i, []).append(mk(emit_outproj_piece, 0, tt, n2))
            attention(1, inj1)

            inj2 = {
                26: [mk(emit_qproj_half, 3, 0, 0)],
                28: [mk(emit_qproj_half, 3, 0, 1)],
                29: [mk(emit_qproj_half, 3, 1, 0)],
                30: [mk(emit_qproj_half, 3, 1, 1)],
            }
            for i, (tt, n2) in enumerate((t, n) for t in range(4) for n in range(2)):
                inj2.setdefault(4 + 3 * i, []).append(mk(emit_outproj_piece, 1, tt, n2))
            attention(2, inj2)

            inj3 = {}
            for i, (tt, n2) in enumerate((t, n) for t in range(4) for n in range(2)):
                inj3.setdefault(4 + 3 * i, []).append(mk(emit_outproj_piece, 2, tt, n2))
            attention(3, inj3)

            for f in pend:
                f()
            for tt in range(4):
                for n2 in range(2):
                    emit_outproj_piece(3, tt, n2, last=True)

    nc.compile()
    return nc


def _get_nc():
    if "nc" not in _CACHE:
        _CACHE["nc"] = _build_nc()
    return _CACHE["nc"]


def kernel(query, key, value, Wq, bq, Wk, bk, Wv, bv, Wo, bo):
    nc = _get_nc()
    scale = np.float32(1.0 / np.sqrt(DK))

    query = np.asarray(query, dtype=np.float32)
    key = np.asarray(key, dtype=np.float32)
    value = np.asarray(value, dtype=np.float32)
    Wq = np.asarray(Wq, dtype=np.float32)
    Wk = np.asarray(Wk, dtype=np.float32)
    Wv = np.asarray(Wv, dtype=np.float32)
    Wo = np.asarray(Wo, dtype=np.float32)

    import ml_dtypes
    F8NP = ml_dtypes.float8_e4m3fn

    def xfmt(x):
        # [S, D] -> [P, NQ5, KC, 512]: partition-major, one contiguous run
        # per (partition, t5) pair
        return np.ascontiguousarray(
            x.T.reshape(KC, P, NQ5, 512).transpose(1, 2, 0, 3).astype(np.float16))

    def xfmt8(x):
        # [S, D] -> [64, NQ5, KC, 2, 512] fp8: DoubleRow splits each 128-row
        # D chunk into two 64-row reduction tiles living in a free dim
        return np.ascontiguousarray(
            x.T.reshape(KC, 2, 64, NQ5, 512).transpose(2, 3, 0, 1, 4).astype(F8NP))

    xq_np = [xfmt8(query[b]) for b in range(B)]
    xk_np = [xfmt8(key[b]) for b in range(B)]
    xv_np = [xfmt(value[b]) for b in range(B)]

    wq_np, wk_np, wv_np, wo_np, bq_np = [], [], [], [], []
    for g in range(GROUPS):
        gsl = slice(CC * g, CC * (g + 1))
        def wfmt(w):
            # [D, CC] -> [P, KC, CC]
            return np.ascontiguousarray(
                w.reshape(KC, P, CC).transpose(1, 0, 2).astype(np.float16))

        def wfmt8(w):
            # [D, CC] -> [64, KC, 2, CC] fp8 (DoubleRow reduction-tile split)
            return np.ascontiguousarray(
                w.reshape(KC, 2, 64, CC).transpose(2, 0, 1, 3).astype(F8NP))

        wq_np.append(wfmt8(Wq[gsl].T))
        wk_np.append(wfmt8(Wk[gsl].T))
        wv_np.append(wfmt(Wv[gsl].T))
        wo_np.append(np.ascontiguousarray(
            Wo[:, gsl].T.reshape(CT, P, D).transpose(1, 0, 2).astype(np.float16)))
        bq_np.append(np.ascontiguousarray((np.asarray(bq, np.float32)[gsl] * scale).reshape(CT, P).T))

    ident_np = np.eye(P, dtype=np.float16)
    in_maps = []
    for c in range(8):
        b, g = c // GROUPS, c % GROUPS
        in_maps.append({
            "xq": xq_np[b], "xk": xk_np[b], "xv": xv_np[b],
            "wq": wq_np[g], "wk": wk_np[g], "wv": wv_np[g],
            "wo": wo_np[g], "bq": bq_np[g], "ident": ident_np,
        })

    res = None
    for attempt in range(3):
        try:
            res = run_bass_kernel_spmd(nc, in_maps, core_ids=list(range(8)))
            _CACHE["last_res"] = res
            break
        except Exception:
            # transient NRT_EXEC_UNIT_UNRECOVERABLE wedge: tear down the PJRT
            # client and retry with a fresh backend connection
            if attempt == 2:
                raise
            import time
            time.sleep(15)
            try:
                import jax
                jax.clear_backends()
            except Exception:
                try:
                    from jax._src import xla_bridge
                    xla_bridge.backends.cache_clear()
                except Exception:
                    pass

    # host combine: sum the 4 head-group partials per batch, add folded bias
    bias = (np.asarray(bo, np.float64)
            + np.asarray(Wo, np.float64) @ np.asarray(bv, np.float64)).astype(np.float32)
    out = np.empty((B, S, D), dtype=np.float32)
    for b in range(B):
        acc = res.results[b * GROUPS + 0]["out"].astype(np.float32)
        for g in range(1, GROUPS):
            acc = acc + res.results[b * GROUPS + g]["out"]
        out[b] = acc + bias
    return out
